# revision 1
# baseline (speedup 1.0000x reference)
"""Trainium2 Bass kernel for nn_MemoryBank (retrieval_knn).

Computes, for each of Q=8192 query embeddings, the minimum Euclidean
distance to any of N=65536 memory-bank rows (D=768).

Strategy (8 NeuronCores):
  - Shard the memory bank rows N across the 8 cores (8192 rows each);
    queries are replicated.
  - Per core: tiles of [128 queries x 512 bank rows]; PSUM accumulates
    -2*q.m over 6 K-chunks of 128 plus one K=2 "ones" matmul that adds
    the (hi/lo bf16 split of the centered) bank-row norms ||m||^2-768.
    A VectorE reduce_min per tile + a final reduce produce
      md2[q] = min_n (||m_n||^2 - 768 - 2 q.m_n)           (one [Q] vector/core)
  - Host: d = sqrt(max(q_sq + 768 + min_over_cores(md2), 0)).

min/sqrt/relu commute, so sqrt and the q-only terms are applied after
the min on the host; all distance-critical accumulation is fp32 (PSUM).
"""

import numpy as np
import ml_dtypes

Q, N, D = 8192, 65536, 768
N_CORES = 8
NS = N // N_CORES          # bank rows per core
NB = 512                   # bank rows per matmul tile (psum free dim)

_BF16 = ml_dtypes.bfloat16

_compiled = None           # cached (nc, ) build


def _build():
    from contextlib import ExitStack
    import concourse.mybir as mybir
    import concourse.tile as tile
    from concourse import bacc

    KD = D // 128
    n_qb = Q // 128
    n_nb = NS // NB

    nc = bacc.Bacc("TRN2", target_bir_lowering=False, debug=False)
    bf16 = mybir.dt.bfloat16
    f32 = mybir.dt.float32

    embT = nc.dram_tensor("embT", [D, Q], bf16, kind="ExternalInput").ap()
    mbT = nc.dram_tensor("mbT", [D, NS], bf16, kind="ExternalInput").ap()
    mbsq2 = nc.dram_tensor("mbsq2", [2, NS], bf16, kind="ExternalInput").ap()
    ones2 = nc.dram_tensor("ones2", [2, 128], bf16, kind="ExternalInput").ap()
    out = nc.dram_tensor("out_md2", [Q], f32, kind="ExternalOutput").ap()

    with tile.TileContext(nc) as tc, ExitStack() as ctx:
        const_pool = ctx.enter_context(tc.tile_pool(name="const", bufs=1))
        emb_pool = ctx.enter_context(tc.tile_pool(name="emb", bufs=1))
        mb_pool = ctx.enter_context(tc.tile_pool(name="mb", bufs=2))
        acc_pool = ctx.enter_context(tc.tile_pool(name="acc", bufs=1))
        fin_pool = ctx.enter_context(tc.tile_pool(name="fin", bufs=4))
        psum_pool = ctx.enter_context(tc.tile_pool(name="ps", bufs=8, space="PSUM"))

        t_ones = const_pool.tile([2, 128], bf16, tag="ones")
        nc.sync.dma_start(t_ones[:, :], ones2[:, :])

        t_emb = []
        for k in range(KD):
            t = emb_pool.tile([128, Q], bf16, tag=f"emb{k}", name=f"emb{k}")
            nc.sync.dma_start(t[:, :], embT[k * 128:(k + 1) * 128, :])
            t_emb.append(t)

        t_acc = [acc_pool.tile([128, n_nb], f32, tag=f"acc{qb}", name=f"acc{qb}")
                 for qb in range(n_qb)]

        for nb in range(n_nb):
            nsl = slice(nb * NB, (nb + 1) * NB)
            t_mb = []
            for k in range(KD):
                t = mb_pool.tile([128, NB], bf16, tag=f"mb{k}", name=f"mb{k}_{nb}")
                nc.sync.dma_start(t[:, :], mbT[k * 128:(k + 1) * 128, nsl])
                t_mb.append(t)
            t_mbsq = mb_pool.tile([2, NB], bf16, tag="mbsq", name=f"mbsq_{nb}")
            nc.sync.dma_start(t_mbsq[:, :], mbsq2[:, nsl])

            for qb in range(n_qb):
                qsl = slice(qb * 128, (qb + 1) * 128)
                ps = psum_pool.tile([128, NB], f32, tag="ps", name=f"ps_{nb}_{qb}")
                nc.tensor.matmul(ps[:, :], t_ones[:, :], t_mbsq[:, :],
                                 start=True, stop=False)
                for k in range(KD):
                    nc.tensor.matmul(ps[:, :], t_emb[k][:, qsl], t_mb[k][:, :],
                                     start=False, stop=(k == KD - 1))
                nc.vector.tensor_reduce(
                    t_acc[qb][:, nb:nb + 1], ps[:, :],
                    axis=mybir.AxisListType.X, op=mybir.AluOpType.min)

        for qb in range(n_qb):
            fin = fin_pool.tile([128, 1], f32, tag="fin", name=f"fin{qb}")
            nc.vector.tensor_reduce(
                fin[:, :], t_acc[qb][:, :],
                axis=mybir.AxisListType.X, op=mybir.AluOpType.min)
            nc.sync.dma_start(out[qb * 128:(qb + 1) * 128], fin[:, :])

    nc.compile()
    return nc


def _get_compiled():
    global _compiled
    if _compiled is None:
        _compiled = _build()
    return _compiled


def _prep_inputs(embeddings, memory_bank):
    """Host-side shard + layout prep. Returns (in_maps, q_sq)."""
    emb = np.asarray(embeddings, dtype=np.float32)
    mb = np.asarray(memory_bank, dtype=np.float32)

    embT = np.ascontiguousarray((-2.0 * emb).T).astype(_BF16)
    q_sq = np.einsum("qd,qd->q", emb.astype(np.float64), emb.astype(np.float64))
    ones2 = np.ones((2, 128), dtype=_BF16)

    in_maps = []
    for c in range(N_CORES):
        shard = mb[c * NS:(c + 1) * NS]
        mbT = np.ascontiguousarray(shard.T).astype(_BF16)
        mb_sq_c = np.einsum(
            "nd,nd->n", shard.astype(np.float64), shard.astype(np.float64)
        ) - float(D)
        hi = mb_sq_c.astype(_BF16)
        lo = (mb_sq_c - hi.astype(np.float64)).astype(_BF16)
        mbsq2 = np.ascontiguousarray(np.stack([hi, lo]))
        in_maps.append({"embT": embT, "mbT": mbT, "mbsq2": mbsq2, "ones2": ones2})
    return in_maps, q_sq


def kernel(embeddings, memory_bank):
    from concourse import bass_utils

    nc = _get_compiled()
    in_maps, q_sq = _prep_inputs(embeddings, memory_bank)
    res = bass_utils.run_bass_kernel_spmd(
        nc, in_maps, core_ids=list(range(N_CORES)))
    md2 = np.min(np.stack([res.results[c]["out_md2"] for c in range(N_CORES)]),
                 axis=0)
    d2 = q_sq + float(D) + md2.astype(np.float64)
    return np.sqrt(np.maximum(d2, 0.0)).astype(np.float32)


# revision 2
# speedup vs baseline: 3.9432x; 3.9432x over previous
"""Trainium2 Bass kernel for nn_MemoryBank (retrieval_knn).

For each of Q=8192 query embeddings, the minimum Euclidean distance to any
of N=65536 memory-bank rows (D=768).

Distribution (8 NeuronCores): memory-bank rows are sharded across cores
(8192 rows each); queries are replicated. Each core computes
    md2[q] = min_n (||m_n||^2 - 2 q.m_n)
over its shard; the host takes the elementwise min across cores, adds
||q||^2, clamps at 0 and takes sqrt (all of which commute with the min).

Per-core kernel layout ("nq"): bank rows live on PSUM partitions, queries
on the free dim. For each tile [128 bank rows x 512 queries]:
  - 6 matmuls (K=128 each) accumulate -2*q.m into PSUM (embT is pre-scaled
    by -2; bf16 inputs, fp32 accumulation),
  - one VectorE scalar_tensor_tensor folds in the row norms and the
    running min:  macc = min(psum + mb_sq[p], macc).
Finale: PE-transpose each 128-column block of macc and reduce_min over the
free dim, giving md2 for 128 queries per block.
"""

from contextlib import ExitStack

import numpy as np
import ml_dtypes

Q, N, D = 8192, 65536, 768
N_CORES = 8
NS = N // N_CORES          # bank rows per core
NQ = 512                   # queries per matmul tile (psum free dim)
WG = 4                     # psum tiles sharing one stationary weight load
GNB = 4                    # stationary blocks fetched per mb DMA
PSUM_BUFS = 8

_BF16 = ml_dtypes.bfloat16
_BIG = 3.0e38

_compiled = None


def _build():
    import concourse.mybir as mybir
    import concourse.tile as tile
    from concourse import bacc
    from concourse.masks import make_identity

    KD = D // 128
    n_nb = NS // 128
    n_qt = Q // NQ

    nc = bacc.Bacc("TRN2", target_bir_lowering=False, debug=False)
    bf16 = mybir.dt.bfloat16
    f32 = mybir.dt.float32

    embT = nc.dram_tensor("embT", [D, Q], bf16, kind="ExternalInput").ap()
    mbT = nc.dram_tensor("mbT", [D, NS], bf16, kind="ExternalInput").ap()
    mbsqT = nc.dram_tensor("mbsqT", [128, n_nb], f32, kind="ExternalInput").ap()
    out = nc.dram_tensor("out_md2", [Q], f32, kind="ExternalOutput").ap()

    with tile.TileContext(nc) as tc, ExitStack() as ctx:
        const_pool = ctx.enter_context(tc.tile_pool(name="const", bufs=1))
        emb_pool = ctx.enter_context(tc.tile_pool(name="emb", bufs=1))
        mb_pool = ctx.enter_context(tc.tile_pool(name="mb", bufs=3))
        macc_pool = ctx.enter_context(tc.tile_pool(name="macc", bufs=1))
        fin_pool = ctx.enter_context(tc.tile_pool(name="fin", bufs=4))
        ps_pool = ctx.enter_context(
            tc.tile_pool(name="ps", bufs=PSUM_BUFS, space="PSUM"))

        ident = const_pool.tile([128, 128], f32, tag="ident")
        make_identity(nc, ident[:, :])

        t_mbsq = const_pool.tile([128, n_nb], f32, tag="mbsq")
        nc.sync.dma_start(t_mbsq[:, :], mbsqT[:, :])

        t_emb = []
        for k in range(KD):
            t = emb_pool.tile([128, Q], bf16, tag=f"emb{k}", name=f"emb{k}")
            step = Q // 4
            for s in range(4):
                nc.sync.dma_start(
                    t[:, s * step:(s + 1) * step],
                    embT[k * 128:(k + 1) * 128, s * step:(s + 1) * step])
            t_emb.append(t)

        t_macc = [macc_pool.tile([128, NQ], mybir.dt.float32, tag=f"macc{qt}",
                                 name=f"macc{qt}")
                  for qt in range(n_qt)]
        for qt in range(n_qt):
            nc.vector.memset(t_macc[qt][:, :], _BIG)

        for nbg in range(n_nb // GNB):
            t_mb = []
            for k in range(KD):
                t = mb_pool.tile([128, GNB * 128], bf16, tag=f"mb{k}",
                                 name=f"mb{k}_{nbg}")
                nc.sync.dma_start(
                    t[:, :], mbT[k * 128:(k + 1) * 128,
                                 nbg * GNB * 128:(nbg + 1) * GNB * 128])
                t_mb.append(t)

            for i in range(GNB):
                nb = nbg * GNB + i
                isl = slice(i * 128, (i + 1) * 128)
                for qt0 in range(0, n_qt, WG):
                    pss = [ps_pool.tile([128, NQ], f32, tag="ps",
                                        name=f"ps_{nb}_{qt}")
                           for qt in range(qt0, qt0 + WG)]
                    for k in range(KD):
                        for j, qt in enumerate(range(qt0, qt0 + WG)):
                            qsl = slice(qt * NQ, (qt + 1) * NQ)
                            nc.tensor.matmul(pss[j][:, :], t_mb[k][:, isl],
                                             t_emb[k][:, qsl],
                                             start=(k == 0),
                                             stop=(k == KD - 1))
                    for j, qt in enumerate(range(qt0, qt0 + WG)):
                        nc.vector.scalar_tensor_tensor(
                            out=t_macc[qt][:, :],
                            in0=pss[j][:, :],
                            scalar=t_mbsq[:, nb:nb + 1],
                            in1=t_macc[qt][:, :],
                            op0=mybir.AluOpType.add,
                            op1=mybir.AluOpType.min)

        for qt in range(n_qt):
            for c in range(NQ // 128):
                pst = ps_pool.tile([128, 128], f32, tag="ps",
                                   name=f"pst_{qt}_{c}")
                nc.tensor.transpose(
                    pst[:, :], t_macc[qt][:, c * 128:(c + 1) * 128],
                    ident[:, :])
                fin = fin_pool.tile([128, 1], f32, tag="fin",
                                    name=f"fin_{qt}_{c}")
                nc.vector.tensor_reduce(
                    fin[:, :], pst[:, :],
                    axis=mybir.AxisListType.X, op=mybir.AluOpType.min)
                q0 = qt * NQ + c * 128
                nc.sync.dma_start(out[q0:q0 + 128], fin[:, :])

    nc.compile()
    return nc


def _get_compiled():
    global _compiled
    if _compiled is None:
        _compiled = _build()
    return _compiled


def _prep_inputs(embeddings, memory_bank):
    emb = np.asarray(embeddings, dtype=np.float32)
    mb = np.asarray(memory_bank, dtype=np.float32)

    embT = np.ascontiguousarray((-2.0 * emb).T).astype(_BF16)
    q_sq = np.einsum("qd,qd->q", emb.astype(np.float64), emb.astype(np.float64))

    in_maps = []
    for c in range(N_CORES):
        shard = mb[c * NS:(c + 1) * NS]
        mbT = np.ascontiguousarray(shard.T).astype(_BF16)
        mb_sq = np.einsum("nd,nd->n", shard.astype(np.float64),
                          shard.astype(np.float64))
        mbsqT = np.ascontiguousarray(
            mb_sq.reshape(-1, 128).T).astype(np.float32)
        in_maps.append({"embT": embT, "mbT": mbT, "mbsqT": mbsqT})
    return in_maps, q_sq


def kernel(embeddings, memory_bank):
    from concourse import bass_utils

    nc = _get_compiled()
    in_maps, q_sq = _prep_inputs(embeddings, memory_bank)
    res = bass_utils.run_bass_kernel_spmd(
        nc, in_maps, core_ids=list(range(N_CORES)))
    md2 = np.min(np.stack([res.results[c]["out_md2"] for c in range(N_CORES)]),
                 axis=0)
    d2 = q_sq + md2.astype(np.float64)
    return np.sqrt(np.maximum(d2, 0.0)).astype(np.float32)
